# Initial kernel scaffold
#
"""MoE expert MLP (SwiGLU, top-2 routing) on 8 Trainium2 NeuronCores.

Strategy: expert-parallel. Host routes tokens (stable argsort by expert id,
matching the reference), gathers each expert's token rows, and pads them to a
fixed capacity C. Core e runs expert e's two GEMMs + SwiGLU over its C-column
token panel; the host scatters results back into the permuted [N, H] output.

Per-core dataflow (all activations kept transposed, tokens on the free dim):
  GEMM1:  h1T[m*128:(m+1)*128, :C] = w1[e][:, mcols].T @ xT   (accumulate over
          8 H k-tiles; w1 k-tiles are the stationary operand in their natural
          [K, M] layout, xT k-tiles are the moving operand)
  SwiGLU: interT[j] = silu(h1T_a[j]) * h1T_b[j]   (a = w1 cols j*128..,
          b = w1 cols 2048 + j*128..), ACT + DVE straight out of PSUM
  GEMM2:  yT[h*128:(h+1)*128, :C] = w2[e][:, hcols].T @ interT (16 I k-tiles)

Weights are re-laid-out on the host so that each DMA lands 128-partition tiles
with multi-KB contiguous per-partition rows, and so that the (a_j, b_j) column
blocks of w1 arrive adjacently and stream in exact consumption order.
"""

import numpy as np
import ml_dtypes

import concourse.bass as bass
import concourse.mybir as mybir
import concourse.tile as tile
from concourse import bacc
from concourse.bass_utils import run_bass_kernel_spmd

BF16 = mybir.dt.bfloat16
F32 = mybir.dt.float32
NP_BF16 = ml_dtypes.bfloat16

# Problem shape (hardcoded per the contract; matches nn_Experts_41429254537622)
B, S, H, I, E, TOPK = 1, 512, 1024, 2048, 8, 2
N_CORES = 8
C = 160          # per-expert token capacity per wave (max observed count is 142)
KH = H // 128    # 8  k-tiles for GEMM1 (contraction over H)
NPAIR = I // 128 # 16 (a, b) pairs of 128-wide w1 column blocks
KI = I // 128    # 16 k-tiles for GEMM2 (contraction over I)
MH = H // 128    # 8  output row blocks of yT
W1_CHUNK = 2     # (a, b) pairs per w1 DMA chunk (1MB)
W2_CHUNK = 2     # h blocks per w2 DMA chunk (1MB)
N_W1C = NPAIR // W1_CHUNK
N_W2C = MH // W2_CHUNK

_compiled = {}
LAST_RUNS = []  # BassKernelResults of the most recent kernel() call (for test harness)


def _build_program():
    nc = bacc.Bacc(
        "TRN2", target_bir_lowering=False, debug=False, num_devices=N_CORES
    )
    # Wide row-major weight tensors sliced per chunk: each 1MB chunk reads 128
    # per-partition rows strided 64KB apart, spreading the reads across HBM
    # banks/channels. (A/B-tested against chunk-contiguous extents — the
    # strided layout sustains higher bandwidth.)
    PAIR_COLS = 2 * KH * 128           # w1 columns per (a, b) pair block
    HB_COLS = KI * 128                 # w2 columns per h block
    xT_d = nc.dram_tensor("xT", [128, KH * C], BF16, kind="ExternalInput")
    w1_d = nc.dram_tensor(
        "w1r", [128, NPAIR * PAIR_COLS], BF16, kind="ExternalInput"
    )
    w2_d = nc.dram_tensor(
        "w2r", [128, MH * HB_COLS], BF16, kind="ExternalInput"
    )
    yT_d = nc.dram_tensor("yT", [128, MH * C], BF16, kind="ExternalOutput")

    # DMA strategy: the SDMA engines round-robin across queues at packet
    # granularity, so spreading large loads over both HWDGE rings makes the
    # EARLIEST chunk finish last (fair sharing) and starves the PE. Instead
    # all loads go on one ring (sync) in exact consumption order — one ring is
    # one HW queue row, so transfers complete FIFO at full rate — with enough
    # slots in the shared weight pool to hide per-transfer issue + completion
    # latency. Stores go on the other HWDGE ring (scalar) so they never
    # head-block the weight stream.
    #
    # The tokens panel and the first w1 pair are loaded as RAW pre-Tile
    # instructions guarded by a manual semaphore, so their data streams
    # concurrently with the framework preamble instead of after it. The wait
    # sits on the CONSUMER (tensor engine, program-order ahead of every
    # Tile-emitted PE instruction) — the sync engine never blocks, flowing
    # straight from these pre-loads into the chunk stream. The sem is cleared
    # after the wait so repeated NEFF executions start from a clean state.
    xt_raw = nc.alloc_sbuf_tensor("xt_pre", [128, KH * C], BF16)
    w1t0_raw = nc.alloc_sbuf_tensor("w1c0_pre", [128, PAIR_COLS], BF16)
    pre_sem = nc.alloc_semaphore(name="pre_dma_sem")
    xt = xt_raw.ap()
    w1t0 = w1t0_raw.ap()
    half = PAIR_COLS // 2
    nc.sync.dma_start(xt[:, :], xT_d[:]).then_inc(pre_sem, 16)
    nc.sync.dma_start(w1t0[:, :half], w1_d[:, :half]).then_inc(pre_sem, 16)
    nc.sync.dma_start(w1t0[:, half:], w1_d[:, half:PAIR_COLS]).then_inc(
        pre_sem, 16
    )
    nc.tensor.wait_ge(pre_sem, 48)
    nc.tensor.sem_clear(pre_sem)

    with tile.TileContext(nc) as tc:
        with (
            tc.tile_pool(name="wp", bufs=12) as wp,
            tc.tile_pool(name="sap", bufs=4) as sap,
            tc.tile_pool(name="itp", bufs=1) as itp,
            tc.tile_pool(name="outp", bufs=2) as outp,
            tc.tile_pool(name="ps1", bufs=6, space="PSUM") as ps1,
            tc.tile_pool(name="ps2", bufs=2, space="PSUM") as ps2,
        ):
            it_all = itp.tile([128, KI * C], BF16)
            # Pair 0 computes from the preamble-loaded raw tiles; pairs 1..15
            # stream as 1MB chunks through the pool.
            chunks = [[0]] + [
                list(range(a, min(a + W1_CHUNK, NPAIR)))
                for a in range(1, NPAIR, W1_CHUNK)
            ]
            for ci, pairs in enumerate(chunks):
                if ci == 0:
                    w1t = w1t0
                else:
                    w1t = wp.tile([128, len(pairs) * PAIR_COLS], BF16, tag="w")
                    nc.sync.dma_start(
                        w1t[:],
                        w1_d[:, pairs[0] * PAIR_COLS:(pairs[-1] + 1) * PAIR_COLS],
                    )
                for jj, j in enumerate(pairs):
                    base = jj * PAIR_COLS
                    pa = ps1.tile([128, C], F32, tag="pab")
                    pb = ps1.tile([128, C], F32, tag="pab")
                    for k in range(KH):
                        nc.tensor.matmul(
                            pa[:],
                            w1t[:, base + k * 128:base + (k + 1) * 128],
                            xt[:, k * C:(k + 1) * C],
                            start=(k == 0),
                            stop=(k == KH - 1),
                        )
                    for k in range(KH):
                        nc.tensor.matmul(
                            pb[:],
                            w1t[:, base + (KH + k) * 128:base + (KH + k + 1) * 128],
                            xt[:, k * C:(k + 1) * C],
                            start=(k == 0),
                            stop=(k == KH - 1),
                        )
                    sa = sap.tile([128, C], F32, tag="sa")
                    nc.scalar.activation(
                        sa[:], pa[:], mybir.ActivationFunctionType.Silu
                    )
                    nc.vector.tensor_mul(it_all[:, j * C:(j + 1) * C], sa[:], pb[:])

            # GEMM2: 1MB w2 chunks (the HBM efficiency knee — 512KB chunks
            # A/B-tested worse) covering 2 h-block accumulation chains each.
            for hc in range(N_W2C):
                w2t = wp.tile([128, W2_CHUNK * HB_COLS], BF16, tag="w")
                nc.sync.dma_start(
                    w2t[:],
                    w2_d[:, hc * W2_CHUNK * HB_COLS:(hc + 1) * W2_CHUNK * HB_COLS],
                )
                # Per-chunk output tile: a shared whole-output tile would make
                # each cast WAR-wait on the previous output DMA.
                yt = outp.tile([128, W2_CHUNK * C], BF16, tag="yt")
                for hh in range(W2_CHUNK):
                    h = hc * W2_CHUNK + hh
                    base = hh * HB_COLS
                    py = ps2.tile([128, C], F32, tag="py")
                    for ki in range(KI):
                        nc.tensor.matmul(
                            py[:],
                            w2t[:, base + ki * 128:base + (ki + 1) * 128],
                            it_all[:, ki * C:(ki + 1) * C],
                            start=(ki == 0),
                            stop=(ki == KI - 1),
                        )
                    nc.vector.tensor_copy(yt[:, hh * C:(hh + 1) * C], py[:])
                nc.scalar.dma_start(
                    yT_d[:, hc * W2_CHUNK * C:(hc + 1) * W2_CHUNK * C], yt[:]
                )
    nc.compile()
    return nc


def _get_program():
    if "nc" not in _compiled:
        _compiled["nc"] = _build_program()
    return _compiled["nc"]


def _relayout_w1(w1_e):
    # w1_e: [H, 2I] bf16 -> [N_W1C, 128, W1_CHUNK*2*KH*128]: chunk ci holds
    # pairs (2ci, 2ci+1); within a pair, a_j's 8 k-tiles then b_j's, each
    # k-tile in stationary [K=128, M=128] layout.
    A = w1_e[:, :I].reshape(H, NPAIR, 128)
    Bh = w1_e[:, I:].reshape(H, NPAIR, 128)
    pairs = np.stack([A, Bh], axis=2)                # [H, NPAIR, 2, 128]
    pairs = pairs.reshape(KH, 128, NPAIR, 2, 128)
    return np.ascontiguousarray(
        pairs.transpose(1, 2, 3, 0, 4).reshape(128, NPAIR * 2 * KH * 128)
    )


def _relayout_w2(w2_e):
    # w2_e: [I, H] bf16 -> [N_W2C, 128, W2_CHUNK*KI*128]; chunk hc holds the
    # KI stationary k-tiles of h blocks (2hc, 2hc+1).
    r = w2_e.reshape(KI, 128, MH, 128)
    return np.ascontiguousarray(
        r.transpose(1, 2, 0, 3).reshape(128, MH * KI * 128)
    )


def kernel(hidden_states, tokens_per_expert, w1, w2):
    x = np.asarray(hidden_states).reshape(-1, H)
    flat = np.asarray(tokens_per_expert).reshape(-1).astype(np.int64)
    w1 = np.asarray(w1)
    w2 = np.asarray(w2)
    n_rows = flat.shape[0]

    order = np.argsort(flat, kind="stable")
    token_of_row = order // TOPK
    counts = np.bincount(flat, minlength=E)
    starts = np.concatenate([[0], np.cumsum(counts)[:-1]])

    x_bf = x.astype(NP_BF16)
    if w1.dtype != NP_BF16:
        w1 = w1.astype(NP_BF16)
    if w2.dtype != NP_BF16:
        w2 = w2.astype(NP_BF16)

    nc = _get_program()
    w1r = [_relayout_w1(w1[e]) for e in range(E)]
    w2r = [_relayout_w2(w2[e]) for e in range(E)]

    out = np.zeros((n_rows, H), dtype=NP_BF16)
    LAST_RUNS.clear()
    n_waves = int(max(1, -(-int(counts.max()) // C)))
    for wave in range(n_waves):
        in_maps = []
        for e in range(E):
            lo = starts[e] + wave * C
            cnt = int(min(C, max(0, counts[e] - wave * C)))
            xe = np.zeros((C, H), dtype=NP_BF16)
            if cnt:
                xe[:cnt] = x_bf[token_of_row[lo:lo + cnt]]
            # xT layout: [128, KH*C], k-tile k at cols [k*C, (k+1)*C):
            # xT[p, k*C + c] = xe[c, k*128 + p]
            xT = np.ascontiguousarray(
                xe.T.reshape(KH, 128, C).transpose(1, 0, 2).reshape(128, KH * C)
            )
            in_maps.append({"xT": xT, "w1r": w1r[e], "w2r": w2r[e]})

        res = run_bass_kernel_spmd(nc, in_maps, list(range(N_CORES)))
        LAST_RUNS.append(res)
        for e in range(E):
            lo = starts[e] + wave * C
            cnt = int(min(C, max(0, counts[e] - wave * C)))
            if not cnt:
                continue
            yT = res.results[e]["yT"]
            # yT[p, h*C + c] = y[c, h*128 + p]
            y = yT.reshape(128, MH, C).transpose(2, 1, 0).reshape(C, H)
            out[lo:lo + cnt] = y[:cnt]
    return out



# revision 1
# speedup vs baseline: 1.9999x; 1.9999x over previous
"""MoE expert MLP (SwiGLU, top-2 routing) on 8 Trainium2 NeuronCores.

Strategy: expert-parallel. Host routes tokens (stable argsort by expert id,
matching the reference), gathers each expert's token rows, and pads them to a
fixed capacity C. Core e runs expert e's two GEMMs + SwiGLU over its C-column
token panel; the host scatters results back into the permuted [N, H] output.

Per-core dataflow (all activations kept transposed, tokens on the free dim):
  GEMM1:  h1T[m*128:(m+1)*128, :C] = w1[e][:, mcols].T @ xT   (accumulate over
          8 H k-tiles; w1 k-tiles are the stationary operand in their natural
          [K, M] layout, xT k-tiles are the moving operand)
  SwiGLU: interT[j] = silu(h1T_a[j]) * h1T_b[j]   (a = w1 cols j*128..,
          b = w1 cols 2048 + j*128..), ACT + DVE straight out of PSUM
  GEMM2:  yT[h*128:(h+1)*128, :C] = w2[e][:, hcols].T @ interT (16 I k-tiles)

Weights are re-laid-out on the host so that each DMA lands 128-partition tiles
with multi-KB contiguous per-partition rows, and so that the (a_j, b_j) column
blocks of w1 arrive adjacently and stream in exact consumption order.
"""

import numpy as np
import ml_dtypes

import concourse.bass as bass
import concourse.mybir as mybir
import concourse.tile as tile
from concourse import bacc
from concourse.bass_utils import run_bass_kernel_spmd

BF16 = mybir.dt.bfloat16
F32 = mybir.dt.float32
NP_BF16 = ml_dtypes.bfloat16

# Problem shape (hardcoded per the contract; matches nn_Experts_41429254537622)
B, S, H, I, E, TOPK = 1, 512, 1024, 2048, 8, 2
N_CORES = 8
C = 160          # per-expert token capacity per wave (max observed count is 142)
KH = H // 128    # 8  k-tiles for GEMM1 (contraction over H)
NPAIR = I // 128 # 16 (a, b) pairs of 128-wide w1 column blocks
KI = I // 128    # 16 k-tiles for GEMM2 (contraction over I)
MH = H // 128    # 8  output row blocks of yT
W1_CHUNK = 2     # (a, b) pairs per w1 DMA chunk (1MB)
W2_CHUNK = 2     # h blocks per w2 DMA chunk (1MB)
N_W1C = NPAIR // W1_CHUNK
N_W2C = MH // W2_CHUNK

_compiled = {}
LAST_RUNS = []  # BassKernelResults of the most recent kernel() call (for test harness)


def _build_program():
    nc = bacc.Bacc(
        "TRN2", target_bir_lowering=False, debug=False, num_devices=N_CORES
    )
    # Wide row-major weight tensors sliced per chunk: each 1MB chunk reads 128
    # per-partition rows strided 64KB apart, spreading the reads across HBM
    # banks/channels. (A/B-tested against chunk-contiguous extents — the
    # strided layout sustains higher bandwidth.)
    PAIR_COLS = 2 * KH * 128           # w1 columns per (a, b) pair block
    HB_COLS = KI * 128                 # w2 columns per h block
    xT_d = nc.dram_tensor("xT", [128, KH * C], BF16, kind="ExternalInput")
    w1_d = nc.dram_tensor(
        "w1r", [128, NPAIR * PAIR_COLS], BF16, kind="ExternalInput"
    )
    w2_d = nc.dram_tensor(
        "w2r", [128, MH * HB_COLS], BF16, kind="ExternalInput"
    )
    yT_d = nc.dram_tensor("yT", [128, MH * C], BF16, kind="ExternalOutput")

    # DMA strategy: the SDMA engines round-robin across queues at packet
    # granularity, so spreading large loads over both HWDGE rings makes the
    # EARLIEST chunk finish last (fair sharing) and starves the PE. Instead
    # all loads go on one ring (sync) in exact consumption order — one ring is
    # one HW queue row, so transfers complete FIFO at full rate — with enough
    # slots in the shared weight pool to hide per-transfer issue + completion
    # latency. Stores go on the other HWDGE ring (scalar) so they never
    # head-block the weight stream.
    #
    # The tokens panel and the first w1 pair are loaded as RAW pre-Tile
    # instructions guarded by a manual semaphore, so their data streams
    # concurrently with the framework preamble instead of after it. The wait
    # sits on the CONSUMER (tensor engine, program-order ahead of every
    # Tile-emitted PE instruction) — the sync engine never blocks, flowing
    # straight from these pre-loads into the chunk stream. The sem is cleared
    # after the wait so repeated NEFF executions start from a clean state.
    xt_raw = nc.alloc_sbuf_tensor("xt_pre", [128, KH * C], BF16)
    w1t0_raw = nc.alloc_sbuf_tensor("w1c0_pre", [128, PAIR_COLS], BF16)
    pre_sem = nc.alloc_semaphore(name="pre_dma_sem")
    xt = xt_raw.ap()
    w1t0 = w1t0_raw.ap()
    half = PAIR_COLS // 2
    nc.sync.dma_start(xt[:, :], xT_d[:]).then_inc(pre_sem, 16)
    nc.sync.dma_start(w1t0[:, :half], w1_d[:, :half]).then_inc(pre_sem, 16)
    nc.sync.dma_start(w1t0[:, half:], w1_d[:, half:PAIR_COLS]).then_inc(
        pre_sem, 16
    )
    nc.tensor.wait_ge(pre_sem, 48)
    nc.tensor.sem_clear(pre_sem)

    with tile.TileContext(nc) as tc:
        with (
            tc.tile_pool(name="wp", bufs=12) as wp,
            tc.tile_pool(name="sap", bufs=4) as sap,
            tc.tile_pool(name="itp", bufs=1) as itp,
            tc.tile_pool(name="outp", bufs=2) as outp,
            tc.tile_pool(name="ps1", bufs=6, space="PSUM") as ps1,
            tc.tile_pool(name="ps2", bufs=2, space="PSUM") as ps2,
        ):
            it_all = itp.tile([128, KI * C], BF16)
            # Pair 0 computes from the preamble-loaded raw tiles; pairs 1..15
            # stream as 1MB chunks through the pool.
            chunks = [[0]] + [
                list(range(a, min(a + W1_CHUNK, NPAIR)))
                for a in range(1, NPAIR, W1_CHUNK)
            ]
            for ci, pairs in enumerate(chunks):
                if ci == 0:
                    w1t = w1t0
                else:
                    w1t = wp.tile([128, len(pairs) * PAIR_COLS], BF16, tag="w")
                    nc.sync.dma_start(
                        w1t[:],
                        w1_d[:, pairs[0] * PAIR_COLS:(pairs[-1] + 1) * PAIR_COLS],
                    )
                for jj, j in enumerate(pairs):
                    base = jj * PAIR_COLS
                    pa = ps1.tile([128, C], F32, tag="pab")
                    pb = ps1.tile([128, C], F32, tag="pab")
                    for k in range(KH):
                        nc.tensor.matmul(
                            pa[:],
                            w1t[:, base + k * 128:base + (k + 1) * 128],
                            xt[:, k * C:(k + 1) * C],
                            start=(k == 0),
                            stop=(k == KH - 1),
                        )
                    for k in range(KH):
                        nc.tensor.matmul(
                            pb[:],
                            w1t[:, base + (KH + k) * 128:base + (KH + k + 1) * 128],
                            xt[:, k * C:(k + 1) * C],
                            start=(k == 0),
                            stop=(k == KH - 1),
                        )
                    sa = sap.tile([128, C], F32, tag="sa")
                    nc.scalar.activation(
                        sa[:], pa[:], mybir.ActivationFunctionType.Silu
                    )
                    nc.vector.tensor_mul(it_all[:, j * C:(j + 1) * C], sa[:], pb[:])

            # GEMM2: 1MB w2 chunks (the HBM efficiency knee — 512KB chunks
            # A/B-tested worse) covering 2 h-block accumulation chains each.
            for hc in range(N_W2C):
                w2t = wp.tile([128, W2_CHUNK * HB_COLS], BF16, tag="w")
                nc.sync.dma_start(
                    w2t[:],
                    w2_d[:, hc * W2_CHUNK * HB_COLS:(hc + 1) * W2_CHUNK * HB_COLS],
                )
                # Per-chunk output tile: a shared whole-output tile would make
                # each cast WAR-wait on the previous output DMA.
                yt = outp.tile([128, W2_CHUNK * C], BF16, tag="yt")
                for hh in range(W2_CHUNK):
                    h = hc * W2_CHUNK + hh
                    base = hh * HB_COLS
                    py = ps2.tile([128, C], F32, tag="py")
                    for ki in range(KI):
                        nc.tensor.matmul(
                            py[:],
                            w2t[:, base + ki * 128:base + (ki + 1) * 128],
                            it_all[:, ki * C:(ki + 1) * C],
                            start=(ki == 0),
                            stop=(ki == KI - 1),
                        )
                    nc.vector.tensor_copy(yt[:, hh * C:(hh + 1) * C], py[:])
                nc.scalar.dma_start(
                    yT_d[:, hc * W2_CHUNK * C:(hc + 1) * W2_CHUNK * C], yt[:]
                )
    nc.compile()
    return nc


def _get_program():
    if "nc" not in _compiled:
        _compiled["nc"] = _build_program()
    return _compiled["nc"]


def _relayout_w1(w1_e):
    # w1_e: [H, 2I] bf16 -> [N_W1C, 128, W1_CHUNK*2*KH*128]: chunk ci holds
    # pairs (2ci, 2ci+1); within a pair, a_j's 8 k-tiles then b_j's, each
    # k-tile in stationary [K=128, M=128] layout.
    A = w1_e[:, :I].reshape(H, NPAIR, 128)
    Bh = w1_e[:, I:].reshape(H, NPAIR, 128)
    pairs = np.stack([A, Bh], axis=2)                # [H, NPAIR, 2, 128]
    pairs = pairs.reshape(KH, 128, NPAIR, 2, 128)
    return np.ascontiguousarray(
        pairs.transpose(1, 2, 3, 0, 4).reshape(128, NPAIR * 2 * KH * 128)
    )


def _relayout_w2(w2_e):
    # w2_e: [I, H] bf16 -> [N_W2C, 128, W2_CHUNK*KI*128]; chunk hc holds the
    # KI stationary k-tiles of h blocks (2hc, 2hc+1).
    r = w2_e.reshape(KI, 128, MH, 128)
    return np.ascontiguousarray(
        r.transpose(1, 2, 0, 3).reshape(128, MH * KI * 128)
    )


def kernel(hidden_states, tokens_per_expert, w1, w2):
    x = np.asarray(hidden_states).reshape(-1, H)
    flat = np.asarray(tokens_per_expert).reshape(-1).astype(np.int64)
    w1 = np.asarray(w1)
    w2 = np.asarray(w2)
    n_rows = flat.shape[0]

    order = np.argsort(flat, kind="stable")
    token_of_row = order // TOPK
    counts = np.bincount(flat, minlength=E)
    starts = np.concatenate([[0], np.cumsum(counts)[:-1]])

    x_bf = x.astype(NP_BF16)
    if w1.dtype != NP_BF16:
        w1 = w1.astype(NP_BF16)
    if w2.dtype != NP_BF16:
        w2 = w2.astype(NP_BF16)

    nc = _get_program()
    w1r = [_relayout_w1(w1[e]) for e in range(E)]
    w2r = [_relayout_w2(w2[e]) for e in range(E)]

    out = np.zeros((n_rows, H), dtype=NP_BF16)
    LAST_RUNS.clear()
    n_waves = int(max(1, -(-int(counts.max()) // C)))
    for wave in range(n_waves):
        in_maps = []
        for e in range(E):
            lo = starts[e] + wave * C
            cnt = int(min(C, max(0, counts[e] - wave * C)))
            xe = np.zeros((C, H), dtype=NP_BF16)
            if cnt:
                xe[:cnt] = x_bf[token_of_row[lo:lo + cnt]]
            # xT layout: [128, KH*C], k-tile k at cols [k*C, (k+1)*C):
            # xT[p, k*C + c] = xe[c, k*128 + p]
            xT = np.ascontiguousarray(
                xe.T.reshape(KH, 128, C).transpose(1, 0, 2).reshape(128, KH * C)
            )
            in_maps.append({"xT": xT, "w1r": w1r[e], "w2r": w2r[e]})

        res = run_bass_kernel_spmd(nc, in_maps, list(range(N_CORES)))
        LAST_RUNS.append(res)
        for e in range(E):
            lo = starts[e] + wave * C
            cnt = int(min(C, max(0, counts[e] - wave * C)))
            if not cnt:
                continue
            yT = res.results[e]["yT"]
            # yT[p, h*C + c] = y[c, h*128 + p]
            y = yT.reshape(128, MH, C).transpose(2, 1, 0).reshape(C, H)
            out[lo:lo + cnt] = y[:cnt]
    return out

